# revision 15
# baseline (speedup 1.0000x reference)
"""BiLSTM + CRF Viterbi decoder (EntModel) on 8 Trainium2 NeuronCores.

Strategy (data-parallel over batch, 8 sentences/core):
 - Host: embedding gather, weight preprocessing (gate prescaling, h'=2h folding,
   bias folding), full-sequence reversal for the backward direction (with an
   exact per-step state reset so alignment becomes length-independent).
 - Device, per core:
     X: bulk input projections xproj = [emb;1] @ [W_ih; b].T  (float32r matmuls)
     S: 512-step fused fwd+bwd LSTM scan; weights stationary (W.T tiles), gate
        dim on partitions so the cell math runs on 128 partitions.
        sigmoid/tanh via a single tanh table: sig(x) = 0.5*(tanh(x/2)+1), with
        g-gate weights pre-scaled by 2 and state stored as c'=2c, h'=2h.
     E: emissions = h' @ (0.5*W_out).T in exact fp32 (bitcast trick on history)
     V: Viterbi delta recurrence (max-plus) on DVE, exports the full delta
        history to DRAM.
 - Host: terminal scores, backpointer recomputation from delta history, and
   backtrace (exact same fp32 adds/argmax the reference does).
"""
import sys
sys.path.insert(0, "/opt/trn_rl_repo")
sys.path.insert(0, "/root/.axon_site/_ro/trn_rl_repo")

import numpy as np

NCORES = 8
B, T, E, HD, KT = 64, 512, 300, 256, 20
BS = B // NCORES          # sentences per core = 8
G = 4 * HD                # gate width = 1024
E1 = E + 1                # embedding + ones row (bias folding)
NT = T * BS               # tokens per core = 4096

_PROG = {}


def _build_program(Tb):
    """Build + compile the 8-core SPMD bass program. Tb = sequence length."""
    from contextlib import ExitStack
    import concourse.bass as bass
    import concourse.tile as tile
    from concourse import bacc, mybir

    dt = mybir.dt
    NTb = Tb * BS
    nc = bacc.Bacc("TRN2", target_bir_lowering=False, debug=False,
                   num_devices=NCORES)

    # ---------------- DRAM I/O ----------------
    def din(name, shape, dty=dt.float32):
        return nc.dram_tensor(name, shape, dty, kind="ExternalInput").ap()

    embT_f = din("embT_f", [E1, NTb], dt.float32r)
    embT_r = din("embT_r", [E1, NTb], dt.float32r)
    wihT_f = din("wihT_f", [E1, G], dt.float32r)
    wihT_b = din("wihT_b", [E1, G], dt.float32r)
    whhT_f = din("whhT_f", [HD, G], dt.float32r)
    whhT_b = din("whhT_b", [HD, G], dt.float32r)
    woT = din("woT", [2 * HD, KT])
    transB = din("transB", [BS, KT * KT])
    valid = din("valid", [BS, Tb], dt.uint8)
    rmask = din("rmask", [128, Tb * BS])
    zt_in = din("zt_in", [128, BS], dt.float32r)

    xp = nc.dram_tensor("xp", [128, Tb, 128], dt.float32).ap()         # xproj [p, t, col]
    emf = nc.dram_tensor("emf", [Tb, BS, KT], dt.float32).ap()         # emis fwd
    emb_ = nc.dram_tensor("emb_", [Tb, BS, KT], dt.float32).ap()       # emis bwd (scan order)
    dh_out = nc.dram_tensor("dh", [BS, Tb, KT], dt.float32,
                            kind="ExternalOutput").ap()

    with tile.TileContext(nc) as tc:
        with ExitStack() as ctx:
            # persistent SBUF state
            hist = nc.alloc_sbuf_tensor("hist", [128, 2, 2, Tb, BS],
                                        dt.float32r).ap()
            whh_sb = nc.alloc_sbuf_tensor("whh_sb", [128, 2, 2, 8, 128],
                                          dt.float32r).ap()  # [p, dir, hc, gc, col]
            rm_sb = nc.alloc_sbuf_tensor("rm_sb", [128, Tb, BS], dt.float32).ap()
            c_st = [nc.alloc_sbuf_tensor(f"c{i}", [128, 2, 2, BS],
                                         dt.float32).ap() for i in range(2)]
            zt = nc.alloc_sbuf_tensor("zt", [128, BS], dt.float32r).ap()
            woT_sb = nc.alloc_sbuf_tensor("woT_sb", [128, 4, KT], dt.float32).ap()
            trb_sb = nc.alloc_sbuf_tensor("trb_sb", [BS, KT * KT], dt.float32).ap()
            val_sb = nc.alloc_sbuf_tensor("val_sb", [BS, Tb], dt.uint8).ap()
            zd = nc.alloc_sbuf_tensor("zd", [BS, KT], dt.float32).ap()

            nc.sync.dma_start(zt[:], zt_in[:])
            nc.vector.memset(zd[:], 0.0)
            for i in range(2):
                nc.vector.memset(c_st[i][:], 0.0)

            # whh load: DRAM [HD=256, 1024] -> [p, dir, hc, gc, 128]
            for d, w in enumerate([whhT_f, whhT_b]):
                for hc in range(2):
                    nc.sync.dma_start(
                        whh_sb[:, d, hc, :, :],
                        w[hc * 128:(hc + 1) * 128, :].rearrange(
                            "p (gc c) -> p gc c", gc=8))
            nc.sync.dma_start(rm_sb[:], rmask[:].rearrange(
                "p (t b) -> p t b", b=BS))
            nc.sync.dma_start(
                woT_sb[:],
                woT[:].rearrange("(c p) k -> p c k", p=128))
            nc.sync.dma_start(trb_sb[:], transB[:])
            nc.sync.dma_start(val_sb[:], valid[:])

            # ---------------- Phase X: input projections ----------------
            ECH = [(0, 128), (128, 128), (256, E1 - 256)]  # 301 = 128+128+45
            with tc.tile_pool(name="xw", bufs=1) as xw_pool, \
                 tc.tile_pool(name="xe", bufs=2) as xe_pool, \
                 tc.tile_pool(name="xst", bufs=3) as xst_pool, \
                 tc.tile_pool(name="xps", bufs=3, space="PSUM") as xps_pool:
                wih_t = {}
                for d, w in enumerate([wihT_f, wihT_b]):
                    for ec, (e0, el) in enumerate(ECH):
                        wt = xw_pool.tile([el, G], dt.float32r,
                                          tag=f"wih_{d}_{ec}")
                        nc.sync.dma_start(wt[:], w[e0:e0 + el, :])
                        wih_t[(d, ec)] = wt
                ntt = NTb // 512
                for d, eT in enumerate([embT_f, embT_r]):
                    for tt in range(ntt):
                        ets = []
                        for ec, (e0, el) in enumerate(ECH):
                            et = xe_pool.tile([el, 512], dt.float32r,
                                              tag=f"et{ec}")
                            nc.sync.dma_start(
                                et[:], eT[e0:e0 + el,
                                          tt * 512:(tt + 1) * 512])
                            ets.append(et)
                        for gc in range(8):
                            ps = xps_pool.tile([128, 512], dt.float32)
                            for ec in range(3):
                                nc.tensor.matmul(
                                    ps[:],
                                    wih_t[(d, ec)][:, gc * 128:(gc + 1) * 128],
                                    ets[ec][:],
                                    start=(ec == 0), stop=(ec == 2))
                            # psum -> sbuf -> xp[p, t, col]
                            st = xst_pool.tile([128, 512], dt.float32,
                                               tag="xstg")
                            nc.vector.tensor_copy(st[:], ps[:])
                            tpb = 512 // BS
                            nc.sync.dma_start(
                                xp[:, tt * tpb:(tt + 1) * tpb,
                                   d * 64 + gc * 8: d * 64 + gc * 8 + BS],
                                st[:].rearrange("p (t b) -> p t b", b=BS))

            # ---------------- Phase S: the LSTM scan ----------------
            TB = min(16, Tb)  # xproj prefetch block
            with tc.tile_pool(name="sxr", bufs=2) as sxr_pool, \
                 tc.tile_pool(name="sgp", bufs=2, space="PSUM") as sgp_pool, \
                 tc.tile_pool(name="stmp", bufs=2) as stmp_pool:
                xr = None
                for t in range(Tb):
                    if t % TB == 0:
                        xr = sxr_pool.tile([128, TB, 128], dt.float32,
                                           tag="xr")
                        nc.sync.dma_start(xr[:], xp[:, t:t + TB, :])
                    ps_g = sgp_pool.tile([128, 128], dt.float32)
                    for d in range(2):
                        for gc in range(8):
                            for hc in range(2):
                                if t == 0:
                                    rhs = zt[:]
                                else:
                                    rhs = hist[:, d, hc, t - 1, :]
                                nc.tensor.matmul(
                                    ps_g[:, d * 64 + gc * 8:
                                         d * 64 + gc * 8 + BS],
                                    whh_sb[:, d, hc, gc, :],
                                    rhs,
                                    start=(hc == 0), stop=(hc == 1))
                    g_sb = stmp_pool.tile([128, 128], dt.float32, tag="g_sb")
                    nc.vector.tensor_add(g_sb[:], ps_g[:], xr[:, t % TB, :])
                    t_all = stmp_pool.tile([128, 128], dt.float32, tag="t_all")
                    nc.scalar.activation(t_all[:], g_sb[:],
                                         mybir.ActivationFunctionType.Tanh,
                                         scale=0.5)
                    # views: cols = (d, gc, b); i: gc0-1, f: gc2-3, g: gc4-5, o: gc6-7
                    tv = t_all[:].rearrange("p (d gc b) -> p d gc b", d=2, b=BS)
                    ti = tv[:, :, 0:2, :]
                    tf = tv[:, :, 2:4, :]
                    tg = tv[:, :, 4:6, :]
                    to = tv[:, :, 6:8, :]
                    c_old = c_st[(t + 1) % 2][:]
                    c_new = c_st[t % 2][:]
                    aa = stmp_pool.tile([128, 2, 2, BS], dt.float32, tag="aa")
                    bb = stmp_pool.tile([128, 2, 2, BS], dt.float32, tag="bb")
                    # a = (tf+1)*c_old ; b = (ti+1)*tg ; c_new = 0.5*a + b
                    nc.vector.scalar_tensor_tensor(
                        aa[:], tf, 1.0, c_old, mybir.AluOpType.add,
                        mybir.AluOpType.mult)
                    nc.vector.scalar_tensor_tensor(
                        bb[:], ti, 1.0, tg, mybir.AluOpType.add,
                        mybir.AluOpType.mult)
                    nc.vector.scalar_tensor_tensor(
                        c_new, aa[:], 0.5, bb[:], mybir.AluOpType.mult,
                        mybir.AluOpType.add)
                    # backward-dir state reset (exact, length-driven);
                    # lengths >= Tb//2 so resets only occur at t < Tb - Tb//2
                    if t <= Tb - Tb // 2 - 1:
                        nc.vector.tensor_mul(
                            c_new[:, 1, :, :], c_new[:, 1, :, :],
                            rm_sb[:, t, :].unsqueeze(1).broadcast_to(
                                [128, 2, BS]))
                    tc_t = stmp_pool.tile([128, 2, 2, BS], dt.float32,
                                          tag="tc_t")
                    nc.scalar.activation(tc_t[:], c_new,
                                         mybir.ActivationFunctionType.Tanh,
                                         scale=0.5)
                    # h' = (to+1)*tanh(c) -> hist[:, :, :, t, :]
                    nc.vector.scalar_tensor_tensor(
                        hist[:, :, :, t, :], to, 1.0, tc_t[:],
                        mybir.AluOpType.add, mybir.AluOpType.mult)

            # ---------------- Phase E: emissions ----------------
            with tc.tile_pool(name="est", bufs=3) as est_pool, \
                 tc.tile_pool(name="eps", bufs=4, space="PSUM") as eps_pool:
                for tt in range(NTb // 128):
                    sl = slice(tt * 128 // BS, (tt + 1) * 128 // BS)
                    for d, dst in enumerate([emf, emb_]):
                        ps = eps_pool.tile([128, KT], dt.float32,
                                           tag=f"eps{d}")
                        for hc in range(2):
                            nc.tensor.matmul(
                                ps[:],
                                hist[:, d, hc, sl, :].rearrange(
                                    "p t b -> p (t b)").bitcast(dt.float32),
                                woT_sb[:, 2 * d + hc, :],
                                start=(hc == 0), stop=(hc == 1))
                        est = est_pool.tile([128, KT], dt.float32,
                                            tag=f"est{d}")
                        nc.vector.tensor_copy(est[:], ps[:])
                        nc.sync.dma_start(dst[sl, :, :], est[:])

            # ---------------- Phase V: Viterbi delta scan ----------------
            VC = min(64, Tb)   # emis chunk
            DC = min(128, Tb)  # delta-history chunk
            with tc.tile_pool(name="vem", bufs=2) as vem_pool, \
                 tc.tile_pool(name="vdh", bufs=2) as vdh_pool, \
                 tc.tile_pool(name="vt", bufs=2) as vt_pool:
                ef_t = eb_t = None
                dh_tiles = [None, None]
                for t in range(Tb):
                    if t % VC == 0:
                        cb = t // VC
                        ef_t = vem_pool.tile([BS, VC, KT], dt.float32,
                                             tag="ef_t")
                        nc.sync.dma_start(
                            ef_t[:], emf[cb * VC:(cb + 1) * VC, :, :]
                            .rearrange("t b k -> b t k"))
                        rb = (Tb // VC) - 1 - cb
                        eb_t = vem_pool.tile([BS, VC, KT], dt.float32,
                                             tag="eb_t")
                        nc.sync.dma_start(
                            eb_t[:], emb_[rb * VC:(rb + 1) * VC, :, :]
                            .rearrange("t b k -> b t k"))
                    if t % DC == 0:
                        dh_tiles[(t // DC) % 2] = vdh_pool.tile(
                            [BS, DC, KT], dt.float32, tag="dh_t",
                            name=f"dh_t_{t // DC}")
                    dh_cur = dh_tiles[(t // DC) % 2]
                    if t == 0:
                        d_prev = zd[:]
                    else:
                        pi = ((t - 1) // DC) % 2
                        d_prev = dh_tiles[pi][:, (t - 1) % DC, :]
                    # scores[b, next, prev] = delta[b, prev] + transB[next, prev]
                    sc = vt_pool.tile([BS, KT, KT], dt.float32, tag="sc")
                    nc.vector.tensor_add(
                        sc[:],
                        d_prev.unsqueeze(1).broadcast_to([BS, KT, KT]),
                        trb_sb[:].rearrange("b (n p) -> b n p", n=KT))
                    m = vt_pool.tile([BS, KT], dt.float32, tag="m")
                    nc.vector.tensor_reduce(m[:], sc[:],
                                            axis=mybir.AxisListType.X,
                                            op=mybir.AluOpType.max)
                    q = vt_pool.tile([BS, KT], dt.float32, tag="q")
                    nc.vector.tensor_add(q[:], m[:], ef_t[:, t % VC, :])
                    dslot = dh_cur[:, t % DC, :]
                    if t < Tb // 2:
                        # always valid: write the new delta straight in
                        nc.vector.tensor_add(
                            dslot, q[:], eb_t[:, (Tb - 1 - t) % VC, :])
                    else:
                        q2 = vt_pool.tile([BS, KT], dt.float32, tag="q2")
                        nc.vector.tensor_add(
                            q2[:], q[:], eb_t[:, (Tb - 1 - t) % VC, :])
                        # delta_t = where(valid, q2, delta_{t-1})
                        nc.vector.tensor_copy(dslot, d_prev)
                        nc.vector.copy_predicated(
                            dslot,
                            val_sb[:, t].unsqueeze(1).broadcast_to([BS, KT]),
                            q2[:])
                    if t % DC == DC - 1:
                        nc.sync.dma_start(
                            dh_out[:, t - DC + 1:t + 1, :], dh_cur[:])

    nc.compile()
    return nc


def _get_prog(Tb):
    if Tb not in _PROG:
        _PROG[Tb] = _build_program(Tb)
    return _PROG[Tb]


def _host_prep(sentence, lengths, embed_table, Wf_ih, Wf_hh, bf, Wb_ih, Wb_hh,
               bb, W_out, b_out, transitions, Tb):
    """Build per-core input maps."""
    sentence = np.asarray(sentence)
    lengths = np.asarray(lengths).astype(np.int64)
    emb = np.asarray(embed_table, np.float32)[sentence]  # [B, T, E]

    gs = np.ones((G,), np.float32)
    gs[2 * HD:3 * HD] = 2.0  # g-gate prescale (tanh single-scale trick)

    def prep_dir(W_ih, W_hh, b):
        wih = np.asarray(W_ih, np.float32) * gs[:, None]
        bias = np.asarray(b, np.float32) * gs
        wihT = np.concatenate([wih.T, bias[None, :]], 0)        # [301, 1024]
        whh = np.asarray(W_hh, np.float32) * gs[:, None] * 0.5  # h'=2h
        return np.ascontiguousarray(wihT), np.ascontiguousarray(whh.T)

    wihT_f, whhT_f = prep_dir(Wf_ih, Wf_hh, bf)
    wihT_b, whhT_b = prep_dir(Wb_ih, Wb_hh, bb)
    woT = np.ascontiguousarray(np.asarray(W_out, np.float32).T * 0.5)
    transB = (np.asarray(transitions, np.float32)
              + np.asarray(b_out, np.float32)[:, None])
    transB_rep = np.ascontiguousarray(
        np.broadcast_to(transB.reshape(1, KT * KT), (BS, KT * KT)))

    in_maps = []
    for c in range(NCORES):
        sl = slice(c * BS, (c + 1) * BS)
        ef = emb[sl]                       # [8, T, E]
        er = ef[:, ::-1, :]                # full reverse
        ones = np.ones((1, Tb * BS), np.float32)

        def to_embT(x):
            xt = np.ascontiguousarray(x.transpose(2, 1, 0)).reshape(E, Tb * BS)
            return np.concatenate([xt, ones], 0)

        L = lengths[sl]
        val = (np.arange(Tb)[None, :] < L[:, None]).astype(np.float32)
        rm = np.ones((Tb, BS), np.float32)
        for b_i in range(BS):
            rs = Tb - int(L[b_i]) - 1
            if rs >= 0:
                rm[rs, b_i] = 0.0
        rm_rep = np.ascontiguousarray(
            np.broadcast_to(rm.reshape(1, Tb * BS), (128, Tb * BS)))
        in_maps.append({
            "embT_f": to_embT(ef), "embT_r": to_embT(er),
            "wihT_f": wihT_f, "wihT_b": wihT_b,
            "whhT_f": whhT_f, "whhT_b": whhT_b,
            "woT": woT, "transB": transB_rep,
            "valid": np.ascontiguousarray(val).astype(np.uint8),
            "rmask": rm_rep,
            "zt_in": np.zeros((128, BS), np.float32),
        })
    return in_maps


def _host_finish(dh_all, lengths, transitions, stop_id, Tb):
    """Terminal scores + backpointer recomputation + backtrace (all fp32)."""
    trans = np.asarray(transitions, np.float32)
    lengths = np.asarray(lengths).astype(np.int64)
    Bn = dh_all.shape[0]
    term = dh_all[:, Tb - 1, :] + trans[int(stop_id)][None, :]
    best = term.argmax(-1).astype(np.int32)
    scores = term.max(-1)
    tags = best.copy()
    prevs = np.zeros((Tb, Bn), np.int32)
    bidx = np.arange(Bn)
    for t in range(Tb - 1, -1, -1):
        d_prev = dh_all[:, t - 1, :] if t > 0 else np.zeros(
            (Bn, KT), np.float32)
        sc = d_prev + trans[tags]          # [B, KT] fp32
        bp = sc.argmax(-1).astype(np.int32)
        v = t < lengths
        tags = np.where(v, bp, tags).astype(np.int32)
        prevs[t] = tags
    paths = np.concatenate([prevs, best[None, :]], 0).T.astype(np.int32)
    return scores.astype(np.float32), paths


def kernel(sentence, lengths, start_id, stop_id, embed_table, Wf_ih, Wf_hh,
           bf, Wb_ih, Wb_hh, bb, W_out, b_out, transitions):
    from concourse.bass_utils import run_bass_kernel_spmd

    Tb = T
    nc = _get_prog(Tb)
    in_maps = _host_prep(sentence, lengths, embed_table, Wf_ih, Wf_hh, bf,
                         Wb_ih, Wb_hh, bb, W_out, b_out, transitions, Tb)
    res = run_bass_kernel_spmd(nc, in_maps, core_ids=list(range(NCORES)))
    dh_all = np.concatenate([res.results[c]["dh"] for c in range(NCORES)], 0)
    return _host_finish(dh_all, lengths, transitions, int(stop_id), Tb)
